# Initial kernel scaffold
#
"""Block-global self-attention Trainium2 kernel (SPMD over 8 NeuronCores).

Sharding: core c -> batch n = c//4, heads h0 = (c%4)*4 .. h0+3.
Each core receives x = hidden[n] [4096,2048] and wq/wk/wv = W[:, cols]
[2048,512], returns out [4096,512] (its head-column stripe of batch n).

Per-core pipeline:
  P: bf16 projections -> qT/kT [d,t] + V2 (t-major, 64-row-shifted so local
     windows are two aligned full-K tiles); fp32 approx q-norms -> grid.
  A: local block attention; softmax without max-subtraction (|score|<8);
     probs kept unnormalized bf16, 1/denom fused into the final ACT copy.
  B: exact top-62 global tokens via: packed-value (quantized norm + token id
     in low mantissa) 3-level max8 top-96 candidates -> indirect-gather
     X rows -> exact fp32 norms -> 62nd threshold (bos/eos forced slots)
     -> sorted final index list -> global attention -> indirect row scatter
     (replace; any duplicate rows carry identical values).
"""
import os
import numpy as np

import concourse.bass as bass
import concourse.bacc as bacc
import concourse.mybir as mybir
from concourse.tile import TileContext, add_dep_helper
from concourse.bass_utils import run_bass_kernel_spmd

F32 = mybir.dt.float32
BF16 = mybir.dt.bfloat16
I32 = mybir.dt.int32

T = 4096
H = 2048
D = 128
NH = 4
KO = H // 128
NB = T // 128
CW = 512
NCHUNK = T // CW
NEG = -30.0
NEGRAW = -30.0 * float(np.sqrt(128.0))  # pre-divided by ACT scale
SCALE = float(1.0 / np.sqrt(128.0))
NCAND = 96
NSLOT = NCAND + 2
NIDX = 66
DEBUG = bool(int(os.environ.get("KERNEL_DEBUG", "0")))


def ts(i, sz):
    return slice(i * sz, (i + 1) * sz)


def _raw(inst):
    return inst.ins if hasattr(inst, "ins") else inst


def build_program():
    nc = bacc.Bacc("TRN2", target_bir_lowering=False, debug=False,
                   enable_asserts=True)
    x_d = nc.dram_tensor("x", (T, H), F32, kind="ExternalInput").ap()
    xt_d = nc.dram_tensor("xt", (H, T), F32, kind="ExternalInput").ap()
    wq_d = nc.dram_tensor("wq", (H, NH * D), F32, kind="ExternalInput").ap()
    wk_d = nc.dram_tensor("wk", (H, NH * D), F32, kind="ExternalInput").ap()
    wv_d = nc.dram_tensor("wv", (H, NH * D), F32, kind="ExternalInput").ap()
    id_d = nc.dram_tensor("ident", (128, 128), F32, kind="ExternalInput").ap()
    out_d = [nc.dram_tensor(f"out{h}", (T, D), F32, kind="ExternalOutput").ap()
             for h in range(NH)]
    dbg = {}
    if DEBUG:
        dbg["na"] = nc.dram_tensor("dbg_na", (128, NH, 32), F32, kind="ExternalOutput").ap()
        dbg["cand"] = nc.dram_tensor("dbg_cand", (NH, NSLOT), F32, kind="ExternalOutput").ap()
        dbg["ne"] = nc.dram_tensor("dbg_ne", (NH, NSLOT), F32, kind="ExternalOutput").ap()
        dbg["sidx"] = nc.dram_tensor("dbg_sidx", (NSLOT, NH), I32, kind="ExternalOutput").ap()

    with TileContext(nc) as tc:
        const = tc.alloc_tile_pool(name="const", bufs=1)
        res = tc.alloc_tile_pool(name="res", bufs=1)
        dram = tc.alloc_tile_pool(name="dram", bufs=1, space="DRAM")

        ident = const.tile([128, 128], F32)
        nc.sync.dma_start(ident[:], id_d)
        identb = const.tile([128, 128], BF16)
        nc.vector.tensor_copy(identb[:], ident[:])
        ones_b = const.tile([128, 1], BF16)
        nc.vector.memset(ones_b[:], 1.0)
        ones = const.tile([128, 1], F32)
        nc.vector.memset(ones[:], 1.0)
        iota_g = const.tile([128, NH, 32], F32)
        nc.gpsimd.iota(iota_g[:], pattern=[[0, NH], [1, 32]], base=0,
                       channel_multiplier=32, allow_small_or_imprecise_dtypes=True)

        qT = [res.tile([128, T], BF16, tag=f"qT{h}", name=f"qT{h}") for h in range(NH)]
        kT = [res.tile([128, 64 + T + 64], BF16, tag=f"kT{h}", name=f"kT{h}") for h in range(NH)]
        V2 = res.tile([128, NB + 1, NH, D + 1], BF16, tag="V2")
        wqb = res.tile([128, KO, NH * D], BF16, tag="wqb")
        nagrid = res.tile([128, NH, 32], F32, tag="nagrid")
        na_dram = dram.tile([NH, T], F32)

        # ---------------- pools ----------------
        psum = tc.alloc_tile_pool(name="psum", bufs=1, space="PSUM")
        ab = tc.alloc_tile_pool(name="ab", bufs=4)

        def psA2k(nm):   # 2KB f32 one-shot psums
            t = psum.tile([128, 512], F32, tag="A2k", bufs=2, name=nm)
            return t
        def psTbf(nm):   # bf16 transpose targets
            t = psum.tile([128, 512], BF16, tag="Tbf", bufs=2, name=nm)
            return t
        def psBLK(nm):   # per-block S + ctx combined (and global Sg)
            t = psum.tile([128, 512], F32, tag="blk", bufs=2, name=nm)
            return t
        def psACC(nm):   # held accumulators
            t = psum.tile([128, 512], F32, tag="ACC", bufs=2, name=nm)
            return t

        # ---------------- interleaved: local attention + global per head ----------------
        out_write_insts = [[] for _ in range(NH)]

        def local_block(h, b):
            blk = psBLK("blk")
            # S^T halves: [tk(128), tq(128)]; half g covers window pos g*128..,
            # i.e. k tokens [b*128 - 64 + g*128, ...). kT is 64-padded.
            for g in range(2):
                seg = b + g
                nc.tensor.matmul(blk[:, g * 128:(g + 1) * 128],
                                 kT[h][:, seg * 128:seg * 128 + 128],
                                 qT[h][:, ts(b, 128)], start=True, stop=True)
            if b == 0:
                nc.vector.memset(blk[0:64, 0:128], NEGRAW)
            if b == NB - 1:
                nc.vector.memset(blk[64:128, 128:256], NEGRAW)
            PT = ab.tile([128, 256], BF16, tag="PT", name="PT", bufs=2)
            nc.scalar.activation(PT[:], blk[:, 0:256], mybir.ActivationFunctionType.Exp,
                                 scale=SCALE)
            pC = blk[:, 256:385]
            nc.tensor.matmul(pC, PT[:, 0:128], V2[:, b, h, :],
                             start=True, stop=False)
            nc.tensor.matmul(pC, PT[:, 128:256], V2[:, b + 1, h, :],
                             start=False, stop=True)
            rc = ab.tile([128, 1], F32, tag="rc", name="rc", bufs=8)
            nc.vector.reciprocal(rc[:], pC[:, 128:129])
            co = ab.tile([128, 128], F32, tag="co", name="co", bufs=3)
            nc.vector.tensor_scalar_mul(co[:], pC[:, 0:128], rc[:])
            w = nc.sync.dma_start(out_d[h][ts(b, 128), :], co[:])
            out_write_insts[h].append(_raw(w))

        def global_head(h):
            qgT = qgTh[h]
            Pg = gbig.tile([128, 64 + T + 64], BF16, tag="Pg", name="Pg", bufs=1)
            nc.vector.memset(Pg[96:128, :], 0.0)
            nc.vector.memset(Pg[0:96, 0:64], 0.0)
            nc.vector.memset(Pg[0:96, 64 + T:], 0.0)
            for j in range(8):
                psg = psBLK("psg")[:NSLOT, :]
                nc.tensor.matmul(psg, qgT[:], kT[h][:, 64 + j * 512:64 + (j + 1) * 512],
                                 start=True, stop=True)
                nc.scalar.activation(Pg[0:NSLOT, 64 + j * 512:64 + (j + 1) * 512], psg,
                                     mybir.ActivationFunctionType.Exp,
                                     scale=SCALE)

            pgc = psACC("pgc")[:NSLOT, :D + 1]
            for j in range(NB + 1):
                ppg = psTbf("ppg")[:, :128]
                nc.tensor.transpose(ppg, Pg[:, j * 128:j * 128 + 128], identb[:])
                pgt = gw.tile([128, 128], BF16, tag="pgt")
                nc.vector.tensor_copy(pgt[:], ppg)
                nc.tensor.matmul(pgc[:], pgt[:, 0:NSLOT], V2[:, j, h, :],
                                 start=(j == 0), stop=(j == NB),
                                 skip_group_check=True)
            rcg = gw.tile([NSLOT, 1], F32, tag="rcg")
            nc.vector.reciprocal(rcg[:], pgc[:, D:D + 1])
            gco = gw.tile([NSLOT, 128], F32, tag="gco")
            nc.vector.tensor_scalar_mul(gco[:], pgc[:, 0:D], rcg[:])
            scat = nc.gpsimd.indirect_dma_start(
                out=out_d[h][:],
                out_offset=bass.IndirectOffsetOnAxis(ap=sidx_i[:, h:h + 1], axis=0),
                in_=gco[:], in_offset=None,
                bounds_check=4095, oob_is_err=False)
            for w in out_write_insts[h]:
                add_dep_helper(_raw(scat), w, reason="scatter after local writes")


        A_DONE = [0]
        # ---------------- phase P ----------------
        na_writes = []
        wkv = tc.alloc_tile_pool(name="wkv", bufs=1)
        wkb = wkv.tile([128, KO, NH * D], BF16, tag="wkb")
        wvb = wkv.tile([128, KO, NH * D], BF16, tag="wvb")
        wb = {"q": wqb, "k": wkb, "v": wvb}

        with tc.tile_pool(name="pp", bufs=2) as pp, \
             tc.tile_pool(name="pp1", bufs=1) as pp1:

            # weights via Pool DMA queue (keeps SP free for x), 256-col slices
            for nm, wd in (("q", wq_d), ("k", wk_d), ("v", wv_d)):
                wr = wd.rearrange("(ko p) m -> p ko m", p=128)
                for kb in range(KO):
                    wstg = pp.tile([128, 1, NH * D], F32, tag="wstg")
                    nc.gpsimd.dma_start(wstg[:], wr[:, kb:kb + 1, :])
                    nc.vector.tensor_copy(wb[nm][:, kb:kb + 1, :], wstg[:])

            for h in range(NH):
                nc.vector.memset(kT[h][:, 0:64], 0.0)
                nc.vector.memset(kT[h][:, 64 + T:], 0.0)
            nc.vector.memset(V2[0:64, 0, :, :], 0.0)
            nc.vector.memset(V2[64:128, NB, :, :], 0.0)
            nc.vector.memset(V2[:, :, :, D:D + 1], 1.0)

            for c in range(NCHUNK):
                xtb = pp1.tile([128, KO, CW], BF16, tag="xtb", bufs=2)
                xtr = xt_d.rearrange("(ko p) t -> p ko t", p=128)
                for kg in range(4):
                    xts = pp.tile([128, 4, CW], F32, tag="xts", bufs=2)
                    nc.sync.dma_start(xts[:], xtr[:, kg * 4:(kg + 1) * 4, ts(c, CW)])
                    nc.vector.tensor_copy(xtb[:, kg * 4:(kg + 1) * 4, :], xts[:])
                for h in range(NH):
                    for nm, dstT in (("q", qT[h]), ("k", kT[h])):
                        ps = psA2k("psqk")
                        for kb in range(KO):
                            nc.tensor.matmul(ps[:], wb[nm][:, kb, ts(h, D)],
                                             xtb[:, kb, :], start=(kb == 0),
                                             stop=(kb == KO - 1))
                        off = 64 if nm == "k" else 0
                        nc.vector.tensor_copy(dstT[:, off + c * CW:off + (c + 1) * CW], ps[:])
                        if nm == "q":
                            sq = pp.tile([128, CW], BF16, tag="sq", bufs=1)
                            nc.vector.tensor_tensor(sq[:], dstT[:, ts(c, CW)],
                                                    dstT[:, ts(c, CW)],
                                                    op=mybir.AluOpType.mult)
                            pn = psA2k("pn")[:1, :]
                            nc.tensor.matmul(pn, ones_b[:], sq[:],
                                             start=True, stop=True)
                            narow = pp.tile([1, CW], F32, tag="narow", bufs=1)
                            nc.vector.tensor_copy(narow[:], pn)
                            w = nc.sync.dma_start(na_dram[h:h + 1, ts(c, CW)], narow[:])
                            na_writes.append(_raw(w))
                for s in range(CW // 128):
                    sg = c * (CW // 128) + s
                    pv = psA2k("psv")
                    for kb in range(KO):
                        nc.tensor.matmul(pv[:], xtb[:, kb, ts(s, 128)],
                                         wb["v"][:, kb, :], start=(kb == 0),
                                         stop=(kb == KO - 1))
                    vt = pp.tile([128, NH * D], BF16, tag="vtmp", bufs=1)
                    nc.vector.tensor_copy(vt[:], pv[:])
                    nc.sync.dma_start(V2[64:128, sg, :, 0:D],
                                      vt[0:64, :].rearrange("p (h d) -> p h d", h=NH))
                    nc.sync.dma_start(V2[0:64, sg + 1, :, 0:D],
                                      vt[64:128, :].rearrange("p (h d) -> p h d", h=NH))
                # interleave ready local-attention blocks (1-chunk lag)
                hi = min(4 * c - 2 + 1, NB)
                for b in range(A_DONE[0], hi):
                    for h in range(NH):
                        local_block(h, b)
                A_DONE[0] = max(A_DONE[0], hi)
        wkv.release()

        # ---------------- phase B part 1: candidates + exact topk ----------------
        gp = tc.alloc_tile_pool(name="gp", bufs=1)
        r = nc.sync.dma_start(nagrid[:],
                              na_dram[:].rearrange("h (p j) -> p h j", p=128))
        for w in na_writes:
            add_dep_helper(_raw(r), w, reason="na grid read after writes")

        m0 = gp.tile([128, NH, 32], F32)
        nc.vector.tensor_scalar(m0[:], iota_g[:], 0.0, scalar2=None,
                                op0=mybir.AluOpType.is_equal)
        m1 = gp.tile([128, NH, 32], F32)
        nc.vector.tensor_scalar(m1[:], iota_g[:], 4095.0, scalar2=None,
                                op0=mybir.AluOpType.is_equal)
        nc.vector.tensor_tensor(m0[:], m0[:], m1[:], op=mybir.AluOpType.add)
        nagp = gp.tile([128, NH, 32], F32)
        nc.vector.tensor_tensor(nagp[:], nagrid[:], m0[:], op=mybir.AluOpType.mult)
        nc.vector.tensor_tensor(nagp[:], nagrid[:], nagp[:], op=mybir.AluOpType.subtract)
        nc.vector.tensor_scalar_mul(m0[:], m0[:], 1.0e6)
        nc.vector.tensor_tensor(nagp[:], nagp[:], m0[:], op=mybir.AluOpType.subtract)
        pk = gp.tile([128, NH, 32], F32)
        nc.vector.tensor_scalar_mul(pk[:], nagp[:], 4.0)
        pki = gp.tile([128, NH, 32], I32)
        nc.vector.tensor_copy(pki[:], pk[:])
        nc.vector.tensor_copy(pk[:], pki[:])
        nc.vector.tensor_scalar_mul(pk[:], pk[:], 0.125)
        io16 = gp.tile([128, NH, 32], F32)
        nc.vector.tensor_scalar_mul(io16[:], iota_g[:], 2.0 ** -16)
        nc.vector.tensor_tensor(pk[:], pk[:], io16[:], op=mybir.AluOpType.add)
        pk2 = pk[:].rearrange("p h j -> p (h j)")

        cand1 = gp.tile([128, NH * 16], F32)
        for h in range(NH):
            for rr in range(2):
                mx = gp.tile([128, 8], F32, tag="mx1")
                nc.vector.max(out=mx[:], in_=pk2[:, ts(h, 32)])
                nc.vector.tensor_copy(cand1[:, h * 16 + rr * 8:h * 16 + rr * 8 + 8], mx[:])
                nc.vector.match_replace(out=pk2[:, ts(h, 32)], in_to_replace=mx[:],
                                        in_values=pk2[:, ts(h, 32)], imm_value=-1e30)
        lvl2 = gp.tile([64, 128], F32)
        for h in range(NH):
            for g in range(8):
                nc.sync.dma_start(lvl2[h * 16:(h + 1) * 16, ts(g, 16)],
                                  cand1[16 * g:16 * (g + 1), ts(h, 16)])
        cand2 = gp.tile([64, 24], F32)
        for rr in range(3):
            mx = gp.tile([64, 8], F32, tag="mx2")
            nc.vector.max(out=mx[:], in_=lvl2[:])
            nc.vector.tensor_copy(cand2[:, ts(rr, 8)], mx[:])
            nc.vector.match_replace(out=lvl2[:], in_to_replace=mx[:],
                                    in_values=lvl2[:], imm_value=-1e30)
        c2d = dram.tile([64, 24], F32)
        w2 = nc.sync.dma_start(c2d[:], cand2[:])
        lvl3 = gp.tile([NH, 384], F32)
        r3 = nc.sync.dma_start(lvl3[:],
                               c2d[:].rearrange("(h p) c -> h (p c)", h=NH))
        add_dep_helper(_raw(r3), _raw(w2), reason="lvl3 read after write")
        tops = gp.tile([NH, NCAND], F32)
        for rr in range(12):
            mx = gp.tile([NH, 8], F32, tag="mx3")
            nc.vector.max(out=mx[:], in_=lvl3[:])
            nc.vector.tensor_copy(tops[:, ts(rr, 8)], mx[:])
            nc.vector.match_replace(out=lvl3[:], in_to_replace=mx[:],
                                    in_values=lvl3[:], imm_value=-1e30)

        def decode_t(dst, src, n):
            t1 = gp.tile([NH, n], F32, tag="dec1")
            nc.vector.tensor_scalar_mul(t1[:], src, 8.0)
            t1i = gp.tile([NH, n], I32, tag="dec2")
            nc.vector.tensor_copy(t1i[:], t1[:])
            t1f = gp.tile([NH, n], F32, tag="dec3")
            nc.vector.tensor_copy(t1f[:], t1i[:])
            nc.vector.tensor_tensor(t1[:], t1[:], t1f[:], op=mybir.AluOpType.subtract)
            nc.vector.tensor_scalar_mul(dst, t1[:], 8192.0)

        cand_t = gp.tile([NH, NSLOT], F32)
        decode_t(cand_t[:, 0:NCAND], tops[:], NCAND)
        nc.vector.memset(cand_t[:, NCAND:NCAND + 1], 0.0)
        nc.vector.memset(cand_t[:, NCAND + 1:NSLOT], 4095.0)
        if DEBUG:
            nc.sync.dma_start(dbg["cand"], cand_t[:])

        # B pools (opened post-P; reuse P space)
        gbig = tc.alloc_tile_pool(name="gbig", bufs=2)
        gw = tc.alloc_tile_pool(name="gw", bufs=2)

        pslt = psA2k("pslt")[:NSLOT, :NH]
        nc.tensor.transpose(pslt, cand_t[:], ident[:NH, :NH])
        ctf = gp.tile([NSLOT, NH], F32)
        nc.vector.tensor_copy(ctf[:], pslt)
        cti = gp.tile([NSLOT, NH], I32)
        nc.vector.tensor_copy(cti[:], ctf[:])

        ne_all = gp.tile([NH, NSLOT], F32)
        qgTh = [None] * NH
        for h in range(NH):
            xsel = gbig.tile([128, H], F32, tag="xsel", bufs=1)
            nc.gpsimd.indirect_dma_start(
                out=xsel[0:NSLOT, :], out_offset=None, in_=x_d,
                in_offset=bass.IndirectOffsetOnAxis(ap=cti[:, h:h + 1], axis=0))
            xct = gbig.tile([128, KO, NSLOT], F32, tag="xct", bufs=1)
            for kb in range(KO):
                ptx = psA2k("ptx")[:, :NSLOT]
                nc.tensor.transpose(ptx, xsel[0:NSLOT, ts(kb, 128)],
                                    ident[:NSLOT, :NSLOT])
                nc.vector.tensor_copy(xct[:, kb, :], ptx)
            pqc = psACC("pqc")[:, :NSLOT]
            for kb in range(KO):
                wqf = gw.tile([128, 1, D], F32, tag="wqf")
                nc.sync.dma_start(
                    wqf[:], wq_d.rearrange("(ko p) m -> p ko m", p=128)[:, kb:kb + 1, ts(h, D)])
                nc.tensor.matmul(pqc, wqf[:, 0, :], xct[:, kb, :],
                                 start=(kb == 0), stop=(kb == KO - 1))
            qcf = gw.tile([128, NSLOT], F32, tag="qcf")
            nc.vector.tensor_copy(qcf[:], pqc)
            qgTh[h] = gbig.tile([128, NSLOT], BF16, tag=f"qgT{h}", name=f"qgT{h}")
            nc.vector.tensor_copy(qgTh[h][:], qcf[:])
            sqc = gw.tile([128, NSLOT], F32, tag="sqc")
            nc.vector.tensor_tensor(sqc[:], qcf[:], qcf[:], op=mybir.AluOpType.mult)
            pne = psA2k("pne")[:1, :NSLOT]
            nc.tensor.matmul(pne, ones[:], sqc[:], start=True, stop=True)
            nerow = gw.tile([1, NSLOT], F32, tag="nerow")
            nc.vector.tensor_copy(nerow[:], pne)
            nc.sync.dma_start(ne_all[h:h + 1, :], nerow[:])
        if DEBUG:
            nc.sync.dma_start(dbg["ne"], ne_all[:])

        ne_work = gp.tile([NH, NSLOT], F32)
        nc.vector.tensor_copy(ne_work[:], ne_all[:])
        tops_e = gp.tile([NH, 64], F32)
        for rr in range(8):
            mx = gp.tile([NH, 8], F32, tag="mxe")
            nc.vector.max(out=mx[:], in_=ne_work[:])
            nc.vector.tensor_copy(tops_e[:, ts(rr, 8)], mx[:])
            nc.vector.match_replace(out=ne_work[:], in_to_replace=mx[:],
                                    in_values=ne_work[:], imm_value=-1e30)
        theta = gp.tile([NH, 1], F32)
        nc.vector.tensor_copy(theta[:], tops_e[:, 61:62])

        # sel over the 98 slots; specials (slots 96/97) always selected
        sel = gp.tile([NH, NSLOT], F32)
        nc.vector.tensor_tensor(sel[:], ne_all[:], theta[:].to_broadcast([NH, NSLOT]),
                                op=mybir.AluOpType.is_ge)
        nc.vector.memset(sel[:, NCAND:NSLOT], 1.0)
        # scatter idx per slot: cand_t if selected else OOB (100000)
        sidx_f = gp.tile([NH, NSLOT], F32)
        nc.vector.tensor_scalar(sidx_f[:], sel[:], -1.0, scalar2=None,
                                op0=mybir.AluOpType.add)
        nc.vector.tensor_scalar_mul(sidx_f[:], sidx_f[:], -100000.0)
        nc.vector.tensor_tensor(sidx_f[:], sidx_f[:], cand_t[:], op=mybir.AluOpType.add)
        p_ = psA2k("ptr")[:NSLOT, :NH]
        nc.tensor.transpose(p_, sidx_f[:], ident[:NH, :NH])
        sf1 = gp.tile([NSLOT, NH], F32)
        nc.vector.tensor_copy(sf1[:], p_)
        sidx_i = gp.tile([NSLOT, NH], I32)
        nc.vector.tensor_copy(sidx_i[:], sf1[:])
        if DEBUG:
            nc.sync.dma_start(dbg["sidx"], sidx_i[:])
            nc.sync.dma_start(dbg["na"], nagrid[:])

        for h in range(NH):
            for b in range(A_DONE[0], NB):
                local_block(h, b)
        for h in range(NH):
            global_head(h)

        gw.release()
        gbig.release()
        gp.release()
        ab.release()
        psum.release()
        dram.release()
        res.release()
        const.release()

    nc.finalize()
    return nc


_NC_CACHE = None


def kernel(**inputs):
    global _NC_CACHE
    hs = np.ascontiguousarray(np.asarray(inputs["hidden_states"], dtype=np.float32))
    Wq = np.ascontiguousarray(np.asarray(inputs["Wq"], dtype=np.float32))
    Wk = np.ascontiguousarray(np.asarray(inputs["Wk"], dtype=np.float32))
    Wv = np.ascontiguousarray(np.asarray(inputs["Wv"], dtype=np.float32))
    ident = np.eye(128, dtype=np.float32)

    if _NC_CACHE is None:
        _NC_CACHE = build_program()
    nc = _NC_CACHE
    xts_host = [np.ascontiguousarray(hs[0].T), np.ascontiguousarray(hs[1].T)]

    in_maps = []
    for c in range(8):
        n = c // 4
        h0 = (c % 4) * NH
        cols = slice(h0 * D, (h0 + NH) * D)
        in_maps.append({
            "x": hs[n],
            "xt": xts_host[n],
            "wq": np.ascontiguousarray(Wq[:, cols]),
            "wk": np.ascontiguousarray(Wk[:, cols]),
            "wv": np.ascontiguousarray(Wv[:, cols]),
            "ident": ident,
        })
    res = run_bass_kernel_spmd(nc, in_maps, core_ids=list(range(8)))
    out = np.zeros((2, T, H), np.float32)
    for c in range(8):
        n = c // 4
        h0 = (c % 4) * NH
        for h in range(NH):
            out[n, :, (h0 + h) * D:(h0 + h + 1) * D] = res.results[c][f"out{h}"]
    return out



# revision 8
# speedup vs baseline: 1.0905x; 1.0905x over previous
"""Block-global self-attention Trainium2 kernel (SPMD over 8 NeuronCores).

Sharding: core c -> batch n = c//4, heads h0 = (c%4)*4 .. h0+3.
Each core receives x = hidden[n] [4096,2048] and wq/wk/wv = W[:, cols]
[2048,512], returns out [4096,512] (its head-column stripe of batch n).

Per-core pipeline:
  P: bf16 projections -> qT/kT [d,t] + V2 (t-major, 64-row-shifted so local
     windows are two aligned full-K tiles); fp32 approx q-norms -> grid.
  A: local block attention; softmax without max-subtraction (|score|<8);
     probs kept unnormalized bf16, 1/denom fused into the final ACT copy.
  B: exact top-62 global tokens via: packed-value (quantized norm + token id
     in low mantissa) 3-level max8 top-96 candidates -> indirect-gather
     X rows -> exact fp32 norms -> 62nd threshold (bos/eos forced slots)
     -> sorted final index list -> global attention -> indirect row scatter
     (replace; any duplicate rows carry identical values).
"""
import os
import numpy as np

import concourse.bass as bass
import concourse.bacc as bacc
import concourse.mybir as mybir
from concourse.tile import TileContext, add_dep_helper
from concourse.bass_utils import run_bass_kernel_spmd

F32 = mybir.dt.float32
BF16 = mybir.dt.bfloat16
I32 = mybir.dt.int32

T = 4096
H = 2048
D = 128
NH = 4
KO = H // 128
NB = T // 128
CW = 512
NCHUNK = T // CW
NEG = -30.0
NEGRAW = -30.0 * float(np.sqrt(128.0))  # pre-divided by ACT scale
SCALE = float(1.0 / np.sqrt(128.0))
NCAND = 96
NSLOT = NCAND + 2
NIDX = 66
DEBUG = bool(int(os.environ.get("KERNEL_DEBUG", "0")))


def ts(i, sz):
    return slice(i * sz, (i + 1) * sz)


def _raw(inst):
    return inst.ins if hasattr(inst, "ins") else inst


def build_program():
    nc = bacc.Bacc("TRN2", target_bir_lowering=False, debug=False,
                   enable_asserts=True)
    x_d = nc.dram_tensor("x", (T, H), F32, kind="ExternalInput").ap()
    xt_d = nc.dram_tensor("xt", (H, T), BF16, kind="ExternalInput").ap()
    wq_d = nc.dram_tensor("wq", (H, NH * D), BF16, kind="ExternalInput").ap()
    wk_d = nc.dram_tensor("wk", (H, NH * D), BF16, kind="ExternalInput").ap()
    wv_d = nc.dram_tensor("wv", (H, NH * D), BF16, kind="ExternalInput").ap()
    wq32_d = nc.dram_tensor("wq32", (H, NH * D), F32, kind="ExternalInput").ap()
    id_d = nc.dram_tensor("ident", (128, 128), F32, kind="ExternalInput").ap()
    out_d = [nc.dram_tensor(f"out{h}", (T, D), F32, kind="ExternalOutput").ap()
             for h in range(NH)]
    dbg = {}
    if DEBUG:
        dbg["na"] = nc.dram_tensor("dbg_na", (128, NH, 32), F32, kind="ExternalOutput").ap()
        dbg["cand"] = nc.dram_tensor("dbg_cand", (NH, NSLOT), F32, kind="ExternalOutput").ap()
        dbg["ne"] = nc.dram_tensor("dbg_ne", (NH, NSLOT), F32, kind="ExternalOutput").ap()
        dbg["sidx"] = nc.dram_tensor("dbg_sidx", (NSLOT, NH), I32, kind="ExternalOutput").ap()

    with TileContext(nc) as tc:
        const = tc.alloc_tile_pool(name="const", bufs=1)
        res = tc.alloc_tile_pool(name="res", bufs=1)
        dram = tc.alloc_tile_pool(name="dram", bufs=1, space="DRAM")

        ident = const.tile([128, 128], F32)
        nc.sync.dma_start(ident[:], id_d)
        identb = const.tile([128, 128], BF16)
        nc.vector.tensor_copy(identb[:], ident[:])
        ones_b = const.tile([128, 1], BF16)
        nc.vector.memset(ones_b[:], 1.0)
        ones = const.tile([128, 1], F32)
        nc.vector.memset(ones[:], 1.0)
        iota_g = const.tile([128, NH, 32], F32)
        nc.gpsimd.iota(iota_g[:], pattern=[[0, NH], [1, 32]], base=0,
                       channel_multiplier=32, allow_small_or_imprecise_dtypes=True)

        qT = [res.tile([128, T], BF16, tag=f"qT{h}", name=f"qT{h}") for h in range(NH)]
        kT = [res.tile([128, 64 + T + 64], BF16, tag=f"kT{h}", name=f"kT{h}") for h in range(NH)]
        V2 = res.tile([128, NB + 1, NH, D + 1], BF16, tag="V2")
        wqb = res.tile([128, KO, NH * D], BF16, tag="wqb")
        nagrid = res.tile([128, NH, 32], F32, tag="nagrid")
        na_dram = dram.tile([NH, T], F32)

        # ---------------- pools ----------------
        psum = tc.alloc_tile_pool(name="psum", bufs=1, space="PSUM")
        ab = tc.alloc_tile_pool(name="ab", bufs=4)

        def psA2k(nm):   # 2KB f32 one-shot psums
            t = psum.tile([128, 512], F32, tag="A2k", bufs=2, name=nm)
            return t
        def psTbf(nm):   # bf16 transpose targets
            t = psum.tile([128, 512], BF16, tag="Tbf", bufs=2, name=nm)
            return t
        def psBLK(nm):   # per-block S + ctx combined (and global Sg)
            t = psum.tile([128, 512], F32, tag="blk", bufs=2, name=nm)
            return t
        def psACC(nm):   # held accumulators
            t = psum.tile([128, 512], F32, tag="ACC", bufs=2, name=nm)
            return t

        # ---------------- interleaved: local attention + global per head ----------------
        out_write_insts = [[] for _ in range(NH)]

        def local_block(h, b):
            blk = psBLK("blk")
            # S^T halves: [tk(128), tq(128)]; half g covers window pos g*128..,
            # i.e. k tokens [b*128 - 64 + g*128, ...). kT is 64-padded.
            for g in range(2):
                seg = b + g
                nc.tensor.matmul(blk[:, g * 128:(g + 1) * 128],
                                 kT[h][:, seg * 128:seg * 128 + 128],
                                 qT[h][:, ts(b, 128)], start=True, stop=True)
            if b == 0:
                nc.vector.memset(blk[0:64, 0:128], NEGRAW)
            if b == NB - 1:
                nc.vector.memset(blk[64:128, 128:256], NEGRAW)
            PT = ab.tile([128, 256], BF16, tag="PT", name="PT", bufs=2)
            nc.scalar.activation(PT[:], blk[:, 0:256], mybir.ActivationFunctionType.Exp,
                                 scale=SCALE)
            pC = blk[:, 256:385]
            nc.tensor.matmul(pC, PT[:, 0:128], V2[:, b, h, :],
                             start=True, stop=False)
            nc.tensor.matmul(pC, PT[:, 128:256], V2[:, b + 1, h, :],
                             start=False, stop=True)
            rc = ab.tile([128, 1], F32, tag="rc", name="rc", bufs=8)
            nc.vector.reciprocal(rc[:], pC[:, 128:129])
            co = ab.tile([128, 128], F32, tag="co", name="co", bufs=3)
            nc.vector.tensor_scalar_mul(co[:], pC[:, 0:128], rc[:])
            w = nc.sync.dma_start(out_d[h][ts(b, 128), :], co[:])
            out_write_insts[h].append(_raw(w))

        def global_head(h):
            qgT = qgTh[h]
            Pg = gbig.tile([128, 64 + T + 64], BF16, tag="Pg", name="Pg", bufs=1)
            nc.vector.memset(Pg[96:128, :], 0.0)
            nc.vector.memset(Pg[0:96, 0:64], 0.0)
            nc.vector.memset(Pg[0:96, 64 + T:], 0.0)
            for j in range(8):
                psg = psBLK("psg")[:NSLOT, :]
                nc.tensor.matmul(psg, qgT[:], kT[h][:, 64 + j * 512:64 + (j + 1) * 512],
                                 start=True, stop=True)
                nc.scalar.activation(Pg[0:NSLOT, 64 + j * 512:64 + (j + 1) * 512], psg,
                                     mybir.ActivationFunctionType.Exp,
                                     scale=SCALE)

            pgc = psACC("pgc")[:NSLOT, :D + 1]
            for j in range(NB + 1):
                ppg = psTbf("ppg")[:, :128]
                nc.tensor.transpose(ppg, Pg[:, j * 128:j * 128 + 128], identb[:])
                pgt = gw.tile([128, 128], BF16, tag="pgt")
                nc.vector.tensor_copy(pgt[:], ppg)
                nc.tensor.matmul(pgc[:], pgt[:, 0:NSLOT], V2[:, j, h, :],
                                 start=(j == 0), stop=(j == NB),
                                 skip_group_check=True)
            rcg = gw.tile([NSLOT, 1], F32, tag="rcg")
            nc.vector.reciprocal(rcg[:], pgc[:, D:D + 1])
            gco = gw.tile([NSLOT, 128], F32, tag="gco")
            nc.vector.tensor_scalar_mul(gco[:], pgc[:, 0:D], rcg[:])
            scat = nc.gpsimd.indirect_dma_start(
                out=out_d[h][:],
                out_offset=bass.IndirectOffsetOnAxis(ap=sidx_i[:, h:h + 1], axis=0),
                in_=gco[:], in_offset=None,
                bounds_check=4095, oob_is_err=False)
            for w in out_write_insts[h]:
                add_dep_helper(_raw(scat), w, reason="scatter after local writes")


        A_DONE = [0]
        # ---------------- phase P ----------------
        na_writes = []
        wkv = tc.alloc_tile_pool(name="wkv", bufs=1)
        wkb = wkv.tile([128, KO, NH * D], BF16, tag="wkb")
        wvb = wkv.tile([128, KO, NH * D], BF16, tag="wvb")
        wb = {"q": wqb, "k": wkb, "v": wvb}

        with tc.tile_pool(name="pp", bufs=2) as pp, \
             tc.tile_pool(name="pp1", bufs=1) as pp1:

            # weights via Pool DMA queue (keeps SP free for x), direct bf16
            for nm, wd in (("q", wq_d), ("k", wk_d), ("v", wv_d)):
                wr = wd.rearrange("(ko p) m -> p ko m", p=128)
                for kb in range(0, KO, 4):
                    nc.gpsimd.dma_start(wb[nm][:, kb:kb + 4, :],
                                        wr[:, kb:kb + 4, :])

            for h in range(NH):
                nc.vector.memset(kT[h][:, 0:64], 0.0)
                nc.vector.memset(kT[h][:, 64 + T:], 0.0)
            nc.vector.memset(V2[0:64, 0, :, :], 0.0)
            nc.vector.memset(V2[64:128, NB, :, :], 0.0)
            nc.vector.memset(V2[:, :, :, D:D + 1], 1.0)

            for c in range(NCHUNK):
                xtb = pp1.tile([128, KO, CW], BF16, tag="xtb", bufs=2)
                xtr = xt_d.rearrange("(ko p) t -> p ko t", p=128)
                for kg in range(2):
                    nc.sync.dma_start(xtb[:, kg * 8:(kg + 1) * 8, :],
                                      xtr[:, kg * 8:(kg + 1) * 8, ts(c, CW)])
                for h in range(NH):
                    for nm, dstT in (("q", qT[h]), ("k", kT[h])):
                        ps = psA2k("psqk")
                        for kb in range(KO):
                            nc.tensor.matmul(ps[:], wb[nm][:, kb, ts(h, D)],
                                             xtb[:, kb, :], start=(kb == 0),
                                             stop=(kb == KO - 1))
                        off = 64 if nm == "k" else 0
                        nc.vector.tensor_copy(dstT[:, off + c * CW:off + (c + 1) * CW], ps[:])
                        if nm == "q":
                            sq = pp.tile([128, CW], BF16, tag="sq", bufs=1)
                            nc.vector.tensor_tensor(sq[:], dstT[:, ts(c, CW)],
                                                    dstT[:, ts(c, CW)],
                                                    op=mybir.AluOpType.mult)
                            pn = psA2k("pn")[:1, :]
                            nc.tensor.matmul(pn, ones_b[:], sq[:],
                                             start=True, stop=True)
                            narow = pp.tile([1, CW], F32, tag="narow", bufs=1)
                            nc.vector.tensor_copy(narow[:], pn)
                            w = nc.sync.dma_start(na_dram[h:h + 1, ts(c, CW)], narow[:])
                            na_writes.append(_raw(w))
                for s in range(CW // 128):
                    sg = c * (CW // 128) + s
                    pv = psA2k("psv")
                    for kb in range(KO):
                        nc.tensor.matmul(pv[:], xtb[:, kb, ts(s, 128)],
                                         wb["v"][:, kb, :], start=(kb == 0),
                                         stop=(kb == KO - 1))
                    vt = pp.tile([128, NH * D], BF16, tag="vtmp", bufs=1)
                    nc.vector.tensor_copy(vt[:], pv[:])
                    nc.sync.dma_start(V2[64:128, sg, :, 0:D],
                                      vt[0:64, :].rearrange("p (h d) -> p h d", h=NH))
                    nc.sync.dma_start(V2[0:64, sg + 1, :, 0:D],
                                      vt[64:128, :].rearrange("p (h d) -> p h d", h=NH))
                # interleave ready local-attention blocks (1-chunk lag)
                hi = min(4 * c - 2 + 1, NB)
                for b in range(A_DONE[0], hi):
                    for h in range(NH):
                        local_block(h, b)
                A_DONE[0] = max(A_DONE[0], hi)
        wkv.release()

        # ---------------- phase B part 1: candidates + exact topk ----------------
        gp = tc.alloc_tile_pool(name="gp", bufs=1)
        r = nc.sync.dma_start(nagrid[:],
                              na_dram[:].rearrange("h (p j) -> p h j", p=128))
        for w in na_writes:
            add_dep_helper(_raw(r), w, reason="na grid read after writes")

        m0 = gp.tile([128, NH, 32], F32)
        nc.vector.tensor_scalar(m0[:], iota_g[:], 0.0, scalar2=None,
                                op0=mybir.AluOpType.is_equal)
        m1 = gp.tile([128, NH, 32], F32)
        nc.vector.tensor_scalar(m1[:], iota_g[:], 4095.0, scalar2=None,
                                op0=mybir.AluOpType.is_equal)
        nc.vector.tensor_tensor(m0[:], m0[:], m1[:], op=mybir.AluOpType.add)
        nagp = gp.tile([128, NH, 32], F32)
        nc.vector.tensor_tensor(nagp[:], nagrid[:], m0[:], op=mybir.AluOpType.mult)
        nc.vector.tensor_tensor(nagp[:], nagrid[:], nagp[:], op=mybir.AluOpType.subtract)
        nc.vector.tensor_scalar_mul(m0[:], m0[:], 1.0e6)
        nc.vector.tensor_tensor(nagp[:], nagp[:], m0[:], op=mybir.AluOpType.subtract)
        pk = gp.tile([128, NH, 32], F32)
        nc.vector.tensor_scalar_mul(pk[:], nagp[:], 4.0)
        pki = gp.tile([128, NH, 32], I32)
        nc.vector.tensor_copy(pki[:], pk[:])
        nc.vector.tensor_copy(pk[:], pki[:])
        nc.vector.tensor_scalar_mul(pk[:], pk[:], 0.125)
        io16 = gp.tile([128, NH, 32], F32)
        nc.vector.tensor_scalar_mul(io16[:], iota_g[:], 2.0 ** -16)
        nc.vector.tensor_tensor(pk[:], pk[:], io16[:], op=mybir.AluOpType.add)
        pk2 = pk[:].rearrange("p h j -> p (h j)")

        cand1 = gp.tile([128, NH * 16], F32)
        for h in range(NH):
            for rr in range(2):
                mx = gp.tile([128, 8], F32, tag="mx1")
                nc.vector.max(out=mx[:], in_=pk2[:, ts(h, 32)])
                nc.vector.tensor_copy(cand1[:, h * 16 + rr * 8:h * 16 + rr * 8 + 8], mx[:])
                nc.vector.match_replace(out=pk2[:, ts(h, 32)], in_to_replace=mx[:],
                                        in_values=pk2[:, ts(h, 32)], imm_value=-1e30)
        lvl2 = gp.tile([64, 128], F32)
        for h in range(NH):
            for g in range(8):
                nc.sync.dma_start(lvl2[h * 16:(h + 1) * 16, ts(g, 16)],
                                  cand1[16 * g:16 * (g + 1), ts(h, 16)])
        cand2 = gp.tile([64, 24], F32)
        for rr in range(3):
            mx = gp.tile([64, 8], F32, tag="mx2")
            nc.vector.max(out=mx[:], in_=lvl2[:])
            nc.vector.tensor_copy(cand2[:, ts(rr, 8)], mx[:])
            nc.vector.match_replace(out=lvl2[:], in_to_replace=mx[:],
                                    in_values=lvl2[:], imm_value=-1e30)
        c2d = dram.tile([64, 24], F32)
        w2 = nc.sync.dma_start(c2d[:], cand2[:])
        lvl3 = gp.tile([NH, 384], F32)
        r3 = nc.sync.dma_start(lvl3[:],
                               c2d[:].rearrange("(h p) c -> h (p c)", h=NH))
        add_dep_helper(_raw(r3), _raw(w2), reason="lvl3 read after write")
        tops = gp.tile([NH, NCAND], F32)
        for rr in range(12):
            mx = gp.tile([NH, 8], F32, tag="mx3")
            nc.vector.max(out=mx[:], in_=lvl3[:])
            nc.vector.tensor_copy(tops[:, ts(rr, 8)], mx[:])
            nc.vector.match_replace(out=lvl3[:], in_to_replace=mx[:],
                                    in_values=lvl3[:], imm_value=-1e30)

        def decode_t(dst, src, n):
            t1 = gp.tile([NH, n], F32, tag="dec1")
            nc.vector.tensor_scalar_mul(t1[:], src, 8.0)
            t1i = gp.tile([NH, n], I32, tag="dec2")
            nc.vector.tensor_copy(t1i[:], t1[:])
            t1f = gp.tile([NH, n], F32, tag="dec3")
            nc.vector.tensor_copy(t1f[:], t1i[:])
            nc.vector.tensor_tensor(t1[:], t1[:], t1f[:], op=mybir.AluOpType.subtract)
            nc.vector.tensor_scalar_mul(dst, t1[:], 8192.0)

        cand_t = gp.tile([NH, NSLOT], F32)
        decode_t(cand_t[:, 0:NCAND], tops[:], NCAND)
        nc.vector.memset(cand_t[:, NCAND:NCAND + 1], 0.0)
        nc.vector.memset(cand_t[:, NCAND + 1:NSLOT], 4095.0)
        if DEBUG:
            nc.sync.dma_start(dbg["cand"], cand_t[:])

        # B pools (opened post-P; reuse P space)
        gbig = tc.alloc_tile_pool(name="gbig", bufs=2)
        gw = tc.alloc_tile_pool(name="gw", bufs=2)
        wq32 = gbig.tile([128, KO, NH * D], F32, tag="wq32", bufs=1)
        wq32r = wq32_d.rearrange("(ko p) m -> p ko m", p=128)
        for kb in range(0, KO, 4):
            nc.gpsimd.dma_start(wq32[:, kb:kb + 4, :], wq32r[:, kb:kb + 4, :])

        pslt = psA2k("pslt")[:NSLOT, :NH]
        nc.tensor.transpose(pslt, cand_t[:], ident[:NH, :NH])
        ctf = gp.tile([NSLOT, NH], F32)
        nc.vector.tensor_copy(ctf[:], pslt)
        cti = gp.tile([NSLOT, NH], I32)
        nc.vector.tensor_copy(cti[:], ctf[:])

        ne_all = gp.tile([NH, NSLOT], F32)
        qgTh = [None] * NH
        for h in range(NH):
            xsel = gbig.tile([128, H], F32, tag="xsel", bufs=1)
            nc.gpsimd.indirect_dma_start(
                out=xsel[0:NSLOT, :], out_offset=None, in_=x_d,
                in_offset=bass.IndirectOffsetOnAxis(ap=cti[:, h:h + 1], axis=0))
            xct = gbig.tile([128, KO, NSLOT], F32, tag="xct", bufs=1)
            for kb in range(KO):
                ptx = psA2k("ptx")[:, :NSLOT]
                nc.tensor.transpose(ptx, xsel[0:NSLOT, ts(kb, 128)],
                                    ident[:NSLOT, :NSLOT])
                nc.vector.tensor_copy(xct[:, kb, :], ptx)
            pqc = psACC("pqc")[:, :NSLOT]
            for kb in range(KO):
                nc.tensor.matmul(pqc, wq32[:, kb, ts(h, D)], xct[:, kb, :],
                                 start=(kb == 0), stop=(kb == KO - 1))
            qcf = gw.tile([128, NSLOT], F32, tag="qcf")
            nc.vector.tensor_copy(qcf[:], pqc)
            qgTh[h] = gbig.tile([128, NSLOT], BF16, tag=f"qgT{h}", name=f"qgT{h}")
            nc.vector.tensor_copy(qgTh[h][:], qcf[:])
            sqc = gw.tile([128, NSLOT], F32, tag="sqc")
            nc.vector.tensor_tensor(sqc[:], qcf[:], qcf[:], op=mybir.AluOpType.mult)
            pne = psA2k("pne")[:1, :NSLOT]
            nc.tensor.matmul(pne, ones[:], sqc[:], start=True, stop=True)
            nerow = gw.tile([1, NSLOT], F32, tag="nerow")
            nc.vector.tensor_copy(nerow[:], pne)
            nc.sync.dma_start(ne_all[h:h + 1, :], nerow[:])
        if DEBUG:
            nc.sync.dma_start(dbg["ne"], ne_all[:])

        ne_work = gp.tile([NH, NSLOT], F32)
        nc.vector.tensor_copy(ne_work[:], ne_all[:])
        tops_e = gp.tile([NH, 64], F32)
        for rr in range(8):
            mx = gp.tile([NH, 8], F32, tag="mxe")
            nc.vector.max(out=mx[:], in_=ne_work[:])
            nc.vector.tensor_copy(tops_e[:, ts(rr, 8)], mx[:])
            nc.vector.match_replace(out=ne_work[:], in_to_replace=mx[:],
                                    in_values=ne_work[:], imm_value=-1e30)
        theta = gp.tile([NH, 1], F32)
        nc.vector.tensor_copy(theta[:], tops_e[:, 61:62])

        # sel over the 98 slots; specials (slots 96/97) always selected
        sel = gp.tile([NH, NSLOT], F32)
        nc.vector.tensor_tensor(sel[:], ne_all[:], theta[:].to_broadcast([NH, NSLOT]),
                                op=mybir.AluOpType.is_ge)
        nc.vector.memset(sel[:, NCAND:NSLOT], 1.0)
        # scatter idx per slot: cand_t if selected else OOB (100000)
        sidx_f = gp.tile([NH, NSLOT], F32)
        nc.vector.tensor_scalar(sidx_f[:], sel[:], -1.0, scalar2=None,
                                op0=mybir.AluOpType.add)
        nc.vector.tensor_scalar_mul(sidx_f[:], sidx_f[:], -100000.0)
        nc.vector.tensor_tensor(sidx_f[:], sidx_f[:], cand_t[:], op=mybir.AluOpType.add)
        p_ = psA2k("ptr")[:NSLOT, :NH]
        nc.tensor.transpose(p_, sidx_f[:], ident[:NH, :NH])
        sf1 = gp.tile([NSLOT, NH], F32)
        nc.vector.tensor_copy(sf1[:], p_)
        sidx_i = gp.tile([NSLOT, NH], I32)
        nc.vector.tensor_copy(sidx_i[:], sf1[:])
        if DEBUG:
            nc.sync.dma_start(dbg["sidx"], sidx_i[:])
            nc.sync.dma_start(dbg["na"], nagrid[:])

        for h in range(NH):
            for b in range(A_DONE[0], NB):
                local_block(h, b)
        for h in range(NH):
            global_head(h)

        gw.release()
        gbig.release()
        gp.release()
        ab.release()
        psum.release()
        dram.release()
        res.release()
        const.release()

    nc.finalize()
    return nc


_NC_CACHE = None


def make_in_maps(inputs):
    import ml_dtypes
    BF = ml_dtypes.bfloat16
    hs = np.ascontiguousarray(np.asarray(inputs["hidden_states"], dtype=np.float32))
    Wq = np.ascontiguousarray(np.asarray(inputs["Wq"], dtype=np.float32))
    Wk = np.ascontiguousarray(np.asarray(inputs["Wk"], dtype=np.float32))
    Wv = np.ascontiguousarray(np.asarray(inputs["Wv"], dtype=np.float32))
    ident = np.eye(128, dtype=np.float32)
    xts_host = [np.ascontiguousarray(hs[0].T.astype(BF)),
                np.ascontiguousarray(hs[1].T.astype(BF))]
    in_maps = []
    for c in range(8):
        n = c // 4
        h0 = (c % 4) * NH
        cols = slice(h0 * D, (h0 + NH) * D)
        in_maps.append({
            "x": hs[n],
            "xt": xts_host[n],
            "wq": np.ascontiguousarray(Wq[:, cols].astype(BF)),
            "wk": np.ascontiguousarray(Wk[:, cols].astype(BF)),
            "wv": np.ascontiguousarray(Wv[:, cols].astype(BF)),
            "wq32": np.ascontiguousarray(Wq[:, cols]),
            "ident": ident,
        })
    return in_maps


def kernel(**inputs):
    global _NC_CACHE
    if _NC_CACHE is None:
        _NC_CACHE = build_program()
    nc = _NC_CACHE
    in_maps = make_in_maps(inputs)
    res = run_bass_kernel_spmd(nc, in_maps, core_ids=list(range(8)))
    out = np.zeros((2, T, H), np.float32)
    for c in range(8):
        n = c // 4
        h0 = (c % 4) * NH
        for h in range(NH):
            out[n, :, (h0 + h) * D:(h0 + h + 1) * D] = res.results[c][f"out{h}"]
    return out

